# revision 3
# baseline (speedup 1.0000x reference)
"""Trainium2 Bass kernel for 2-layer GCN (nn_BasicGNN).

Strategy (8 NeuronCores, SPMD):
  - out[v] = dinv[v] * (sum_{u->v} dinv[u]*h[u] + dinv[v]*h[v])  (self-loop)
    => per-node pre-scale by dinv, aggregate raw sums, post-scale by dinv.
  - Layer 2 reordered as (A_norm @ z1) @ W2 so both aggregations move 16 feats.
  - Shard destinations across 8 cores (12500 each). Host sorts each core's
    dests by in-degree, pads edge lists per 128-dest group to the group max
    K_g, and maps edge sources to table rows (per-core slabs concatenated by
    AllGather, bf16).
  - Gather via per-column indirect DMA (128 descriptors each) — the Q7
    descriptor-generation rate (~8ns/desc) is the hard bottleneck, so
    everything else (reduces, scaling, self terms, slab IO) is batched into
    big chunks that hide behind the Pool-engine gather stream:
    * gathers land in multi-group chunk tiles (3-deep pipeline),
    * one DVE reduce per group, elementwise ops once per chunk,
    * self terms kept SBUF-resident, slab writes chunked 3D-AP DMAs,
    * output accumulated in SBUF, single DMA at the end.
"""

import sys
import numpy as np

if "/opt/trn_rl_repo" not in sys.path:
    sys.path.insert(0, "/opt/trn_rl_repo")

N_CORES = 8
P = 128
CHUNK_COLS = 288    # max gather columns per chunk tile


def _to_bf16(a):
    import ml_dtypes
    return np.asarray(a, dtype=np.float32).astype(ml_dtypes.bfloat16)


def _preprocess(x, edge_index, W1, b1, W2, b2):
    x = np.asarray(x, dtype=np.float32)
    W1 = np.asarray(W1, dtype=np.float32)
    b1 = np.asarray(b1, dtype=np.float32)
    W2 = np.asarray(W2, dtype=np.float32)
    b2 = np.asarray(b2, dtype=np.float32)
    N, F_IN = x.shape
    F_HID = W1.shape[1]
    F_OUT = W2.shape[1]
    M = N_CORES
    assert N % M == 0
    Ns = N // M
    NsP = ((Ns + P - 1) // P) * P      # padded dest count per core
    G = NsP // P                        # dest groups per core
    S = NsP + P                         # slab rows (last P rows = zeros)
    PAD_ROW = NsP                       # table row guaranteed zero (core 0)

    row = np.asarray(edge_index[0]).astype(np.int64)
    col = np.asarray(edge_index[1]).astype(np.int64)
    deg = np.bincount(col, minlength=N).astype(np.int64) + 1
    dinv = (deg.astype(np.float64) ** -0.5).astype(np.float32)

    # Per-core degree-sorted dest permutation; node -> global table row
    pos_global = np.empty(N, dtype=np.int64)
    pos_of_list = []
    sorted_indeg = []
    for m in range(M):
        indeg = deg[m * Ns:(m + 1) * Ns] - 1
        key = np.concatenate([indeg, np.full(NsP - Ns, -1, dtype=np.int64)])
        order = np.argsort(key, kind="stable")
        pos_of = np.empty(NsP, dtype=np.int64)
        pos_of[order] = np.arange(NsP)
        pos_of_list.append(pos_of)
        sorted_indeg.append(np.maximum(key[order], 0))
        pos_global[m * Ns:(m + 1) * Ns] = m * S + pos_of[:Ns]

    # Shared per-group K (max over cores, >=1)
    Ks = np.zeros(G, dtype=np.int64)
    for m in range(M):
        si = sorted_indeg[m].reshape(G, P)
        Ks = np.maximum(Ks, si.max(axis=1))
    Ks = np.maximum(Ks, 1)
    offs = np.zeros(G + 1, dtype=np.int64)
    offs[1:] = np.cumsum(Ks)
    SUMK = int(offs[-1])

    # Pack consecutive groups into chunks with total cols <= CHUNK_COLS
    chunks = []  # (g0, Gc, C0)
    g0 = 0
    while g0 < G:
        Gc = 1
        tot = int(Ks[g0])
        while g0 + Gc < G and tot + int(Ks[g0 + Gc]) <= CHUNK_COLS:
            tot += int(Ks[g0 + Gc])
            Gc += 1
        chunks.append((g0, Gc, int(offs[g0])))
        g0 += Gc

    in_maps = []
    for m in range(M):
        pos_of = pos_of_list[m]
        mask = (col >= m * Ns) & (col < (m + 1) * Ns)
        er = row[mask]
        dpos = pos_of[col[mask] - m * Ns]
        src_row = pos_global[er]
        o = np.argsort(dpos, kind="stable")
        dpos = dpos[o]
        src_row = src_row[o]
        cnt = np.bincount(dpos, minlength=NsP)
        starts = np.concatenate([[0], np.cumsum(cnt)])[:-1]
        rank = np.arange(len(dpos)) - starts[dpos]
        idx_all = np.full((P, SUMK), PAD_ROW, dtype=np.int32)
        g_of = dpos // P
        p_of = dpos % P
        idx_all[p_of, offs[g_of] + rank] = src_row.astype(np.int32)

        xp = np.zeros((NsP, F_IN), np.float32)
        xp[pos_of[:Ns]] = x[m * Ns:(m + 1) * Ns]
        xT = np.ascontiguousarray(xp.T)

        d_sorted = np.ones(NsP, np.float32)
        d_sorted[pos_of[:Ns]] = dinv[m * Ns:(m + 1) * Ns]
        dgp = d_sorted.reshape(G, P).T                      # [P, G]
        dinvb = np.ascontiguousarray(
            np.repeat(dgp[:, :, None], F_HID, axis=2).reshape(P, G * F_HID))
        dinv2b = np.ascontiguousarray(dinvb * dinvb)
        b1b = np.ascontiguousarray(
            np.tile(b1[None, None, :], (P, G, 1)).reshape(P, G * F_HID))

        in_maps.append({
            "xT": _to_bf16(xT),
            "idx": idx_all,
            "dinv": np.ascontiguousarray(dgp),
            "dinvb": dinvb.astype(np.float32),
            "dinv2b": dinv2b.astype(np.float32),
            "b1b": b1b.astype(np.float32),
            "W1": _to_bf16(W1),
            "W2": W2,
            "b2r": np.ascontiguousarray(np.tile(b2[None, :], (P, 1))),
        })

    meta = dict(N=N, Ns=Ns, NsP=NsP, G=G, S=S, SUMK=SUMK,
                Ks=Ks.tolist(), offs=offs.tolist(), chunks=chunks,
                F_IN=F_IN, F_HID=F_HID, F_OUT=F_OUT,
                pos_of_list=pos_of_list, b1_zero=bool(not np.any(b1)))
    return meta, in_maps


def _build_program(meta):
    import concourse.bacc as bacc
    import concourse.tile as tile
    import concourse.bass as bass
    import concourse.mybir as mybir
    from concourse.masks import make_identity

    f32 = mybir.dt.float32
    bf16 = mybir.dt.bfloat16
    i32 = mybir.dt.int32
    G, S, NsP = meta["G"], meta["S"], meta["NsP"]
    SUMK = meta["SUMK"]
    Ks, offs, chunks = meta["Ks"], meta["offs"], meta["chunks"]
    F_IN, F_HID, F_OUT = meta["F_IN"], meta["F_HID"], meta["F_OUT"]
    b1_zero = meta["b1_zero"]
    M = N_CORES
    X = mybir.AxisListType.X

    nc = bacc.Bacc("TRN2", target_bir_lowering=False, debug=False,
                   enable_asserts=False, num_devices=M)

    xT_d = nc.dram_tensor("xT", [P, NsP], bf16, kind="ExternalInput")
    idx_d = nc.dram_tensor("idx", [P, SUMK], i32, kind="ExternalInput")
    dinv_d = nc.dram_tensor("dinv", [P, G], f32, kind="ExternalInput")
    dinvb_d = nc.dram_tensor("dinvb", [P, G * F_HID], f32, kind="ExternalInput")
    dinv2b_d = nc.dram_tensor("dinv2b", [P, G * F_HID], f32, kind="ExternalInput")
    b1b_d = nc.dram_tensor("b1b", [P, G * F_HID], f32, kind="ExternalInput")
    W1_d = nc.dram_tensor("W1", [F_IN, F_HID], bf16, kind="ExternalInput")
    W2_d = nc.dram_tensor("W2", [F_HID, F_OUT], f32, kind="ExternalInput")
    b2r_d = nc.dram_tensor("b2r", [P, F_OUT], f32, kind="ExternalInput")
    out_d = nc.dram_tensor("out", [NsP, F_OUT], f32, kind="ExternalOutput")

    slab1 = nc.dram_tensor("slab1", [S, F_HID], bf16, kind="Internal")
    slab2 = nc.dram_tensor("slab2", [S, F_HID], bf16, kind="Internal")
    tab1 = nc.dram_tensor("tab1", [M * S, F_HID], bf16, kind="Internal",
                          addr_space="Shared")
    tab2 = nc.dram_tensor("tab2", [M * S, F_HID], bf16, kind="Internal",
                          addr_space="Shared")
    RG = [list(range(M))]

    with tile.TileContext(nc) as tc:
        with tc.tile_pool(name="big", bufs=1) as bigp, \
             tc.tile_pool(name="wts", bufs=1) as wp, \
             tc.tile_pool(name="gath", bufs=3) as gp, \
             tc.tile_pool(name="yt", bufs=2) as ytp, \
             tc.tile_pool(name="ps1", bufs=2, space="PSUM") as pp1, \
             tc.tile_pool(name="ps2", bufs=2, space="PSUM") as pp2, \
             tc.tile_pool(name="ps3", bufs=2, space="PSUM") as pp3:

            xT_s = bigp.tile([P, NsP], bf16)
            nc.sync.dma_start(xT_s[:], xT_d[:])
            idx_s = bigp.tile([P, SUMK], i32)
            nc.sync.dma_start(idx_s[:], idx_d[:])
            dinv_s = wp.tile([P, G], f32)
            nc.sync.dma_start(dinv_s[:], dinv_d[:])
            dinvb_s = wp.tile([P, G * F_HID], f32)
            nc.sync.dma_start(dinvb_s[:], dinvb_d[:])
            dinv2b_s = wp.tile([P, G * F_HID], f32)
            nc.sync.dma_start(dinv2b_s[:], dinv2b_d[:])
            if not b1_zero:
                b1b_s = wp.tile([P, G * F_HID], f32)
                nc.sync.dma_start(b1b_s[:], b1b_d[:])
            W1_s = wp.tile([F_IN, F_HID], bf16)
            nc.sync.dma_start(W1_s[:], W1_d[:])
            W2_s = wp.tile([F_HID, F_OUT], f32)
            nc.sync.dma_start(W2_s[:], W2_d[:])
            b2r_s = wp.tile([P, F_OUT], f32)
            nc.sync.dma_start(b2r_s[:], b2r_d[:])
            ident = wp.tile([P, P], f32)
            make_identity(nc, ident[:])
            zt = wp.tile([P, F_HID], bf16)
            nc.vector.memset(zt[:], 0.0)
            nc.sync.dma_start(slab1[NsP:NsP + P, :], zt[:])
            nc.sync.dma_start(slab2[NsP:NsP + P, :], zt[:])

            t1_all = bigp.tile([P, G, F_HID], bf16)
            t2_all = bigp.tile([P, G, F_HID], bf16)
            s1_all = bigp.tile([P, G, F_HID], f32)
            s2_all = bigp.tile([P, G, F_HID], f32)
            y_all = bigp.tile([P, G, F_HID], f32)
            out_all = bigp.tile([P, G, F_OUT], f32)

            # ---- Phase A: t1 = dinv * (x @ W1)  (bf16 table entries) ----
            for g in range(G):
                ps = pp1.tile([P, F_HID], f32, tag="mm1")
                nc.tensor.matmul(ps[:], lhsT=xT_s[:, g * P:(g + 1) * P],
                                 rhs=W1_s[:], start=True, stop=True)
                nc.scalar.mul(t1_all[:, g, :], ps[:], dinv_s[:, g:g + 1])
            nc.sync.dma_start(
                slab1[0:NsP, :].rearrange("(g p) f -> p g f", p=P), t1_all[:])

            nc.gpsimd.collective_compute(
                "AllGather", mybir.AluOpType.bypass, replica_groups=RG,
                ins=[slab1[:]], outs=[tab1[:]])

            def gather_chunk(tab, g0, Gc, C0):
                cols = int(offs[g0 + Gc]) - C0
                gt = gp.tile([P, CHUNK_COLS, F_HID], bf16, tag="gt")
                for c in range(cols):
                    nc.gpsimd.indirect_dma_start(
                        out=gt[:, c, :], out_offset=None, in_=tab[:],
                        in_offset=bass.IndirectOffsetOnAxis(
                            ap=idx_s[:, C0 + c:C0 + c + 1], axis=0))
                return gt

            def reduce_chunk(gt, s_all, g0, Gc, C0):
                for g in range(g0, g0 + Gc):
                    a = int(offs[g]) - C0
                    nc.vector.reduce_sum(
                        out=s_all[:, g, :],
                        in_=gt[:, a:a + int(Ks[g]), :].rearrange("p k f -> p f k"),
                        axis=X)

            # ---- Layer 1 aggregation + pointwise ----
            for (g0, Gc, C0) in chunks:
                gt = gather_chunk(tab1, g0, Gc, C0)
                reduce_chunk(gt, s1_all, g0, Gc, C0)
                sl1 = s1_all[:, g0:g0 + Gc, :]
                tl = t2_all[:, g0:g0 + Gc, :]
                a = g0 * F_HID
                b = (g0 + Gc) * F_HID
                nc.vector.tensor_add(sl1, sl1, t1_all[:, g0:g0 + Gc, :])
                if b1_zero:
                    nc.vector.tensor_mul(sl1, sl1, dinv2b_s[:, a:b])
                    nc.vector.tensor_scalar_max(tl, sl1, 0.0)
                else:
                    nc.vector.tensor_mul(sl1, sl1, dinvb_s[:, a:b])
                    nc.vector.tensor_add(sl1, sl1, b1b_s[:, a:b])
                    nc.vector.tensor_scalar_max(sl1, sl1, 0.0)
                    nc.vector.tensor_mul(tl, sl1, dinvb_s[:, a:b])
                nc.sync.dma_start(
                    slab2[g0 * P:(g0 + Gc) * P, :].rearrange(
                        "(g p) f -> p g f", p=P), tl)

            nc.gpsimd.collective_compute(
                "AllGather", mybir.AluOpType.bypass, replica_groups=RG,
                ins=[slab2[:]], outs=[tab2[:]])

            # ---- Layer 2 aggregation + output transform ----
            for (g0, Gc, C0) in chunks:
                gt = gather_chunk(tab2, g0, Gc, C0)
                reduce_chunk(gt, s2_all, g0, Gc, C0)
                sl2 = s2_all[:, g0:g0 + Gc, :]
                nc.vector.tensor_add(sl2, sl2, t2_all[:, g0:g0 + Gc, :])
                nc.vector.tensor_mul(y_all[:, g0:g0 + Gc, :], sl2,
                                     dinvb_s[:, g0 * F_HID:(g0 + Gc) * F_HID])
                for g in range(g0, g0 + Gc):
                    tp = pp2.tile([F_HID, P], f32, tag="tr")
                    nc.tensor.transpose(tp[:], y_all[:, g, :], ident[:])
                    yT = ytp.tile([F_HID, P], f32, tag="yT")
                    nc.scalar.copy(yT[:], tp[:])
                    op = pp3.tile([P, F_OUT], f32, tag="mm2")
                    nc.tensor.matmul(op[:], lhsT=yT[:], rhs=W2_s[:],
                                     start=True, stop=True)
                    nc.vector.tensor_add(out_all[:, g, :], op[:], b2r_s[:])

            nc.sync.dma_start(
                out_d[0:NsP, :].rearrange("(g p) f -> p g f", p=P), out_all[:])

    nc.compile()
    return nc


def _assemble(results, meta):
    M = N_CORES
    Ns, N, F_OUT = meta["Ns"], meta["N"], meta["F_OUT"]
    out = np.empty((N, F_OUT), dtype=np.float32)
    for m in range(M):
        pos_of = meta["pos_of_list"][m]
        out[m * Ns:(m + 1) * Ns] = results[m]["out"][pos_of[:Ns]]
    return out


_CACHE = {}


def kernel(x, edge_index, W1, b1, W2, b2):
    meta, in_maps = _preprocess(x, edge_index, W1, b1, W2, b2)
    key = (meta["N"], meta["SUMK"], tuple(meta["Ks"]), meta["b1_zero"])
    if key not in _CACHE:
        _CACHE[key] = _build_program(meta)
    nc = _CACHE[key]
    from concourse import bass_utils
    res = bass_utils.run_bass_kernel_spmd(nc, in_maps, core_ids=list(range(N_CORES)))
    return _assemble(res.results, meta)


# revision 8
# speedup vs baseline: 1.0006x; 1.0006x over previous
"""Trainium2 Bass kernel for 2-layer GCN (nn_BasicGNN).

Strategy (8 NeuronCores, SPMD):
  - out[v] = dinv[v] * (sum_{u->v} dinv[u]*h[u] + dinv[v]*h[v])  (self-loop)
    => per-node pre-scale by dinv, aggregate raw sums, post-scale by dinv.
  - Layer 2 reordered as (A_norm @ z1) @ W2 so both aggregations move 16 feats.
  - Shard destinations across 8 cores (12500 each). Host sorts each core's
    dests by in-degree, pads edge lists per 128-dest group to the group max
    K_g, and maps edge sources to table rows (per-core slabs concatenated by
    AllGather, bf16).
  - Gather via per-column indirect DMA (128 descriptors each) — the Q7
    descriptor-generation rate (~8ns/desc) is the hard bottleneck, so
    everything else (reduces, scaling, self terms, slab IO) is batched into
    big chunks that hide behind the Pool-engine gather stream:
    * gathers land in multi-group chunk tiles (3-deep pipeline),
    * one DVE reduce per group, elementwise ops once per chunk,
    * self terms kept SBUF-resident, slab writes chunked 3D-AP DMAs,
    * output accumulated in SBUF, single DMA at the end.
"""

import sys
import numpy as np

if "/opt/trn_rl_repo" not in sys.path:
    sys.path.insert(0, "/opt/trn_rl_repo")

N_CORES = 8
P = 128
CHUNK_COLS = 288    # max gather columns per chunk tile


def _to_bf16(a):
    import ml_dtypes
    return np.asarray(a, dtype=np.float32).astype(ml_dtypes.bfloat16)


def _preprocess(x, edge_index, W1, b1, W2, b2):
    x = np.asarray(x, dtype=np.float32)
    W1 = np.asarray(W1, dtype=np.float32)
    b1 = np.asarray(b1, dtype=np.float32)
    W2 = np.asarray(W2, dtype=np.float32)
    b2 = np.asarray(b2, dtype=np.float32)
    N, F_IN = x.shape
    F_HID = W1.shape[1]
    F_OUT = W2.shape[1]
    M = N_CORES
    assert N % M == 0
    Ns = N // M
    NsP = ((Ns + P - 1) // P) * P      # padded dest count per core
    G = NsP // P                        # dest groups per core
    S = NsP + P                         # slab rows (last P rows = zeros)

    row = np.asarray(edge_index[0]).astype(np.int64)
    col = np.asarray(edge_index[1]).astype(np.int64)
    deg = np.bincount(col, minlength=N).astype(np.int64) + 1
    dinv = (deg.astype(np.float64) ** -0.5).astype(np.float32)

    GH = (NsP // P) // 2
    RH = GH * P                         # slab rows in first table segment
    # Per-core degree-sorted dest permutation; node -> global table row.
    # Table layout: [all cores' slab rows 0:RH][all cores' slab rows RH:S]
    pos_global = np.empty(N, dtype=np.int64)
    pos_of_list = []
    sorted_indeg = []
    for m in range(M):
        indeg = deg[m * Ns:(m + 1) * Ns] - 1
        key = np.concatenate([indeg, np.full(NsP - Ns, -1, dtype=np.int64)])
        order = np.argsort(key, kind="stable")
        pos_of = np.empty(NsP, dtype=np.int64)
        pos_of[order] = np.arange(NsP)
        pos_of_list.append(pos_of)
        sorted_indeg.append(np.maximum(key[order], 0))
        pg = np.where(pos_of[:Ns] < RH,
                      m * RH + pos_of[:Ns],
                      M * RH + m * (S - RH) + (pos_of[:Ns] - RH))
        pos_global[m * Ns:(m + 1) * Ns] = pg

    # Shared per-group K (max over cores, >=1)
    Ks = np.zeros(G, dtype=np.int64)
    for m in range(M):
        si = sorted_indeg[m].reshape(G, P)
        Ks = np.maximum(Ks, si.max(axis=1))
    Ks = np.maximum(Ks, 1)
    offs = np.zeros(G + 1, dtype=np.int64)
    offs[1:] = np.cumsum(Ks)
    SUMK = int(offs[-1])

    # Pack consecutive groups into chunks with total cols <= CHUNK_COLS
    chunks = []  # (g0, Gc, C0)
    g0 = 0
    while g0 < G:
        Gc = 1
        tot = int(Ks[g0])
        while g0 + Gc < G and tot + int(Ks[g0 + Gc]) <= CHUNK_COLS:
            tot += int(Ks[g0 + Gc])
            Gc += 1
        chunks.append((g0, Gc, int(offs[g0])))
        g0 += Gc

    in_maps = []
    for m in range(M):
        pos_of = pos_of_list[m]
        mask = (col >= m * Ns) & (col < (m + 1) * Ns)
        er = row[mask]
        dpos = pos_of[col[mask] - m * Ns]
        src_row = pos_global[er]
        o = np.argsort(dpos, kind="stable")
        dpos = dpos[o]
        src_row = src_row[o]
        PAD_ROW = M * RH + (NsP - RH)   # core-0 zero-block row, segment 2
        cnt = np.bincount(dpos, minlength=NsP)
        starts = np.concatenate([[0], np.cumsum(cnt)])[:-1]
        rank = np.arange(len(dpos)) - starts[dpos]
        idx_all = np.full((P, SUMK), PAD_ROW, dtype=np.int32)
        g_of = dpos // P
        p_of = dpos % P
        idx_all[p_of, offs[g_of] + rank] = src_row.astype(np.int32)

        xp = np.zeros((NsP, F_IN), np.float32)
        xp[pos_of[:Ns]] = x[m * Ns:(m + 1) * Ns]
        xT = np.ascontiguousarray(xp.T)

        d_sorted = np.ones(NsP, np.float32)
        d_sorted[pos_of[:Ns]] = dinv[m * Ns:(m + 1) * Ns]
        dgp = d_sorted.reshape(G, P).T                      # [P, G]
        dinvb = np.ascontiguousarray(
            np.repeat(dgp[:, :, None], F_HID, axis=2).reshape(P, G * F_HID))
        dinv2b = np.ascontiguousarray(dinvb * dinvb)
        b1b = np.ascontiguousarray(
            np.tile(b1[None, None, :], (P, G, 1)).reshape(P, G * F_HID))

        in_maps.append({
            "xT": _to_bf16(xT),
            "idx": idx_all,
            "dinv": np.ascontiguousarray(dgp),
            "dinvb": _to_bf16(dinvb),
            "dinv2b": _to_bf16(dinv2b),
            "b1b": b1b.astype(np.float32),
            "W1": _to_bf16(W1),
            "W2": W2,
            "b2r": np.ascontiguousarray(np.tile(b2[None, :], (P, 1))),
        })

    meta = dict(N=N, Ns=Ns, NsP=NsP, G=G, S=S, GH=GH, RH=RH, SUMK=SUMK,
                Ks=Ks.tolist(), offs=offs.tolist(), chunks=chunks,
                F_IN=F_IN, F_HID=F_HID, F_OUT=F_OUT,
                pos_of_list=pos_of_list, b1_zero=bool(not np.any(b1)))
    return meta, in_maps


def _build_program(meta):
    import concourse.bacc as bacc
    import concourse.tile as tile
    import concourse.bass as bass
    import concourse.mybir as mybir
    from concourse.masks import make_identity

    f32 = mybir.dt.float32
    bf16 = mybir.dt.bfloat16
    i32 = mybir.dt.int32
    G, S, NsP = meta["G"], meta["S"], meta["NsP"]
    SUMK = meta["SUMK"]
    Ks, offs, chunks = meta["Ks"], meta["offs"], meta["chunks"]
    F_IN, F_HID, F_OUT = meta["F_IN"], meta["F_HID"], meta["F_OUT"]
    b1_zero = meta["b1_zero"]
    M = N_CORES
    X = mybir.AxisListType.X

    nc = bacc.Bacc("TRN2", target_bir_lowering=False, debug=False,
                   enable_asserts=False, num_devices=M)

    xT_d = nc.dram_tensor("xT", [P, NsP], bf16, kind="ExternalInput")
    idx_d = nc.dram_tensor("idx", [P, SUMK], i32, kind="ExternalInput")
    dinv_d = nc.dram_tensor("dinv", [P, G], f32, kind="ExternalInput")
    dinvb_d = nc.dram_tensor("dinvb", [P, G * F_HID], bf16, kind="ExternalInput")
    dinv2b_d = nc.dram_tensor("dinv2b", [P, G * F_HID], bf16, kind="ExternalInput")
    b1b_d = nc.dram_tensor("b1b", [P, G * F_HID], f32, kind="ExternalInput")
    W1_d = nc.dram_tensor("W1", [F_IN, F_HID], bf16, kind="ExternalInput")
    W2_d = nc.dram_tensor("W2", [F_HID, F_OUT], f32, kind="ExternalInput")
    b2r_d = nc.dram_tensor("b2r", [P, F_OUT], f32, kind="ExternalInput")
    out_d = nc.dram_tensor("out", [NsP, F_OUT], f32, kind="ExternalOutput")

    slab1 = nc.dram_tensor("slab1", [S, F_HID], bf16, kind="Internal")
    slab2 = nc.dram_tensor("slab2", [S, F_HID], bf16, kind="Internal")
    tab1 = nc.dram_tensor("tab1", [M * S, F_HID], bf16, kind="Internal",
                          addr_space="Shared")
    tab2 = nc.dram_tensor("tab2", [M * S, F_HID], bf16, kind="Internal",
                          addr_space="Shared")
    RG = [list(range(M))]

    with tile.TileContext(nc) as tc:
        with tc.tile_pool(name="big", bufs=1) as bigp, \
             tc.tile_pool(name="wts", bufs=1) as wp, \
             tc.tile_pool(name="gath", bufs=3) as gp, \
             tc.tile_pool(name="yt", bufs=2) as ytp, \
             tc.tile_pool(name="ps1", bufs=2, space="PSUM") as pp1, \
             tc.tile_pool(name="ps2", bufs=2, space="PSUM") as pp2, \
             tc.tile_pool(name="ps3", bufs=2, space="PSUM") as pp3:

            xT_s = bigp.tile([P, NsP], bf16)
            nc.sync.dma_start(xT_s[:], xT_d[:])
            idx_s = bigp.tile([P, SUMK], i32)
            nc.sync.dma_start(idx_s[:], idx_d[:])
            dinv_s = wp.tile([P, G], f32)
            nc.sync.dma_start(dinv_s[:], dinv_d[:])
            dinvb_s = wp.tile([P, G * F_HID], bf16)
            nc.sync.dma_start(dinvb_s[:], dinvb_d[:])
            dinv2b_s = wp.tile([P, G * F_HID], bf16)
            nc.sync.dma_start(dinv2b_s[:], dinv2b_d[:])
            if not b1_zero:
                b1b_s = wp.tile([P, G * F_HID], f32)
                nc.sync.dma_start(b1b_s[:], b1b_d[:])
            W1_s = wp.tile([F_IN, F_HID], bf16)
            nc.sync.dma_start(W1_s[:], W1_d[:])
            W2_s = wp.tile([F_HID, F_OUT], f32)
            nc.sync.dma_start(W2_s[:], W2_d[:])
            b2r_s = wp.tile([P, F_OUT], f32)
            nc.sync.dma_start(b2r_s[:], b2r_d[:])
            ident = wp.tile([P, P], f32)
            make_identity(nc, ident[:])
            zt = wp.tile([P, F_HID], bf16)
            nc.vector.memset(zt[:], 0.0)
            nc.sync.dma_start(slab1[NsP:NsP + P, :], zt[:])
            nc.sync.dma_start(slab2[NsP:NsP + P, :], zt[:])

            t1_all = bigp.tile([P, G, F_HID], bf16)
            t2_all = bigp.tile([P, G, F_HID], bf16)
            s1_all = bigp.tile([P, G, F_HID], f32)
            s2_all = bigp.tile([P, G, F_HID], f32)
            y_all = bigp.tile([P, G, F_HID], f32)
            out_all = bigp.tile([P, G, F_OUT], f32)

            # ---- Phase A: t1 = dinv * (x @ W1)  (bf16 table entries) ----
            GH, RH = meta["GH"], meta["RH"]

            def collective_on(eng, ins, outs):
                nc.gpsimd.collective_compute(
                    "AllGather", mybir.AluOpType.bypass,
                    replica_groups=RG, ins=ins, outs=outs)

            for g in range(G):
                ps = pp1.tile([P, F_HID], f32, tag="mm1")
                nc.tensor.matmul(ps[:], lhsT=xT_s[:, g * P:(g + 1) * P],
                                 rhs=W1_s[:], start=True, stop=True)
                nc.scalar.mul(t1_all[:, g, :], ps[:], dinv_s[:, g:g + 1])
                if g == GH - 1:
                    nc.sync.dma_start(
                        slab1[0:RH, :].rearrange("(g p) f -> p g f", p=P),
                        t1_all[:, 0:GH, :])
                    collective_on(nc.vector, [slab1[0:RH, :]],
                                  [tab1[0:M * RH, :]])
            nc.sync.dma_start(
                slab1[RH:NsP, :].rearrange("(g p) f -> p g f", p=P),
                t1_all[:, GH:G, :])
            collective_on(nc.vector, [slab1[RH:S, :]],
                          [tab1[M * RH:M * S, :]])

            def gather_chunk(tab, g0, Gc, C0):
                cols = int(offs[g0 + Gc]) - C0
                gt = gp.tile([P, CHUNK_COLS, F_HID], bf16, tag="gt")
                for c in range(cols):
                    nc.gpsimd.indirect_dma_start(
                        out=gt[:, c, :], out_offset=None, in_=tab[:],
                        in_offset=bass.IndirectOffsetOnAxis(
                            ap=idx_s[:, C0 + c:C0 + c + 1], axis=0))
                return gt

            def reduce_chunk(gt, s_all, g0, Gc, C0):
                for g in range(g0, g0 + Gc):
                    a = int(offs[g]) - C0
                    nc.vector.reduce_sum(
                        out=s_all[:, g, :],
                        in_=gt[:, a:a + int(Ks[g]), :].rearrange("p k f -> p f k"),
                        axis=X)

            # ---- Layer 1 aggregation + pointwise ----
            for (g0, Gc, C0) in chunks:
                gt = gather_chunk(tab1, g0, Gc, C0)
                reduce_chunk(gt, s1_all, g0, Gc, C0)
                sl1 = s1_all[:, g0:g0 + Gc, :]
                tl = t2_all[:, g0:g0 + Gc, :]
                a = g0 * F_HID
                b = (g0 + Gc) * F_HID
                nc.vector.tensor_add(sl1, sl1, t1_all[:, g0:g0 + Gc, :])
                if b1_zero:
                    nc.vector.tensor_mul(sl1, sl1, dinv2b_s[:, a:b])
                    nc.vector.tensor_scalar_max(tl, sl1, 0.0)
                else:
                    nc.vector.tensor_mul(sl1, sl1, dinvb_s[:, a:b])
                    nc.vector.tensor_add(sl1, sl1, b1b_s[:, a:b])
                    nc.vector.tensor_scalar_max(sl1, sl1, 0.0)
                    nc.vector.tensor_mul(tl, sl1, dinvb_s[:, a:b])
                nc.sync.dma_start(
                    slab2[g0 * P:(g0 + Gc) * P, :].rearrange(
                        "(g p) f -> p g f", p=P), tl)
                if g0 < GH <= g0 + Gc:
                    collective_on(nc.vector, [slab2[0:RH, :]],
                                  [tab2[0:M * RH, :]])

            collective_on(nc.vector, [slab2[RH:S, :]],
                          [tab2[M * RH:M * S, :]])

            # ---- Layer 2 aggregation + output transform ----
            for (g0, Gc, C0) in chunks:
                gt = gather_chunk(tab2, g0, Gc, C0)
                reduce_chunk(gt, s2_all, g0, Gc, C0)
                sl2 = s2_all[:, g0:g0 + Gc, :]
                nc.vector.tensor_add(sl2, sl2, t2_all[:, g0:g0 + Gc, :])
                nc.vector.tensor_mul(y_all[:, g0:g0 + Gc, :], sl2,
                                     dinvb_s[:, g0 * F_HID:(g0 + Gc) * F_HID])
                for g in range(g0, g0 + Gc):
                    tp = pp2.tile([F_HID, P], f32, tag="tr")
                    nc.tensor.transpose(tp[:], y_all[:, g, :], ident[:])
                    yT = ytp.tile([F_HID, P], f32, tag="yT")
                    nc.scalar.copy(yT[:], tp[:])
                    op = pp3.tile([P, F_OUT], f32, tag="mm2")
                    nc.tensor.matmul(op[:], lhsT=yT[:], rhs=W2_s[:],
                                     start=True, stop=True)
                    nc.vector.tensor_add(out_all[:, g, :], op[:], b2r_s[:])

            nc.sync.dma_start(
                out_d[0:NsP, :].rearrange("(g p) f -> p g f", p=P), out_all[:])

    nc.compile()
    return nc


def _assemble(results, meta):
    M = N_CORES
    Ns, N, F_OUT = meta["Ns"], meta["N"], meta["F_OUT"]
    out = np.empty((N, F_OUT), dtype=np.float32)
    for m in range(M):
        pos_of = meta["pos_of_list"][m]
        out[m * Ns:(m + 1) * Ns] = results[m]["out"][pos_of[:Ns]]
    return out


_CACHE = {}


def kernel(x, edge_index, W1, b1, W2, b2):
    meta, in_maps = _preprocess(x, edge_index, W1, b1, W2, b2)
    key = (meta["N"], meta["SUMK"], tuple(meta["Ks"]), meta["b1_zero"])
    if key not in _CACHE:
        _CACHE[key] = _build_program(meta)
    nc = _CACHE[key]
    from concourse import bass_utils
    res = bass_utils.run_bass_kernel_spmd(nc, in_maps, core_ids=list(range(N_CORES)))
    return _assemble(res.results, meta)
